# revision 28
# baseline (speedup 1.0000x reference)
"""Trainium2 Bass kernel for nn_AttentionReweightingFusion.

Contract: kernel(**inputs) takes FULL (unsharded) numpy inputs as produced by
setup_inputs() and returns the FULL [16384, 1024] float32 output.

Strategy (pure data parallel over 8 NeuronCores, weights replicated):
  - 2048 batch rows per core, processed in 4 tiles of 512 rows with
    software-pipelined emission: attention of tile t-1, the z-chain of
    tile t, and the combine/transposes of tile t+1 are interleaved so all
    five engines stay busy (per-engine queues are strict FIFO in emission
    order; sequential per-tile emission serializes the whole pipeline).
  - The two 512x512 difficulty-compensation GEMMs run in fp8e4 with
    DoubleRow perf mode (two K-chunks per pass, half the matmul count).
    Weights are pre-scaled x64 on chip so their ~0.02-scale entries sit
    in the e4m3 normal range; activation reads apply scale=1/64.
  - The collapsed attention GEMM (wc = wv@wo; softmax over one key is 1)
    stays bf16: fp8 there fails the 2e-2 gate.
  - Features load as bf16 via SWDGE cast-DMA; combine/gating/output math
    runs on 16-bit operands. Output is written bf16, upcast on host.
  - Engine balance: PE matmuls+transposes; ACT gelu/sigmoid/out-scale +
    fp8 evacuations; DVE combine/t1/ct + bf16 evacuations; GPSIMD only
    issues cast-DMAs (its elementwise ops measured ~8x slower than DVE
    and poison DVE via SBUF port contention).
  - The identity matrix and the difficulty row arrive as extra host
    inputs, keeping the rank-1 z1 term and the transposes off the
    quality-transpose critical path.
  - Per-row scalar math (missing-type coefficients, thresholds) is exact
    fp32, matching the reference's branch decisions bit-exactly.
  - A dependency-free warmup matmul burst plus filler matmuls through
    the prologue hold the PE HAM clock gate at 8/8 from t~4us.
"""

import numpy as np

H = 512
B_FULL = 16384
N_CORES = 8
B_CORE = B_FULL // N_CORES          # 2048
TILE_N = 512                        # batch rows per compute tile
N_TILES = B_CORE // TILE_N          # 4
PC = H // 128                       # feature chunks of 128 (4)
RC_TOT = B_CORE // 128              # row chunks per core (16)

_CACHE: dict = {}

# Exposed for test.py after a profiled run
last_exec_time_ns = None
last_trace_path = None
last_scope_times = None


def _interleave(*lists):
    """Proportional round-robin merge preserving each list's order."""
    lists = [list(l) for l in lists if l]
    idx = [0] * len(lists)
    out = []
    while True:
        best, bestfrac = -1, -1.0
        for i, l in enumerate(lists):
            if idx[i] < len(l):
                frac = (len(l) - idx[i]) / len(l)
                if frac > bestfrac:
                    best, bestfrac = i, frac
        if best < 0:
            return out
        out.append(lists[best][idx[best]])
        idx[best] += 1


def _build_program(use_bvo=True):
    from contextlib import ExitStack

    import concourse.bacc as bacc
    import concourse.mybir as mybir
    import concourse.tile as tile

    dt = mybir.dt
    f32 = dt.float32
    bf16 = dt.bfloat16
    fp8 = dt.float8e4
    AF = mybir.ActivationFunctionType
    OP = mybir.AluOpType
    DR = mybir.MatmulPerfMode.DoubleRow

    nc = bacc.Bacc(num_swdge_queues=4)

    # ---------------- DRAM I/O (per-core shapes) ----------------
    d_img = nc.dram_tensor("image_feat", [B_CORE, H], f32, kind="ExternalInput")
    d_txt = nc.dram_tensor("text_feat", [B_CORE, H], f32, kind="ExternalInput")
    d_eimg = nc.dram_tensor("enhanced_image_feat", [B_CORE, H], f32, kind="ExternalInput")
    d_etxt = nc.dram_tensor("enhanced_text_feat", [B_CORE, H], f32, kind="ExternalInput")
    d_qual = nc.dram_tensor("quality", [B_CORE, 11], f32, kind="ExternalInput")
    d_miss = nc.dram_tensor("missing_f", [B_CORE], f32, kind="ExternalInput")

    d_qa_w1 = nc.dram_tensor("qa_w1", [11, 64], f32, kind="ExternalInput")
    d_qa_b1 = nc.dram_tensor("qa_b1", [64], f32, kind="ExternalInput")
    d_qa_w2 = nc.dram_tensor("qa_w2", [64, 32], f32, kind="ExternalInput")
    d_qa_b2 = nc.dram_tensor("qa_b2", [32], f32, kind="ExternalInput")
    d_qa_w3 = nc.dram_tensor("qa_w3", [32, 1], f32, kind="ExternalInput")
    d_qa_b3 = nc.dram_tensor("qa_b3", [1], f32, kind="ExternalInput")
    d_mi_w1 = nc.dram_tensor("mi_w1", [4, 32], f32, kind="ExternalInput")
    d_mi_b1 = nc.dram_tensor("mi_b1", [32], f32, kind="ExternalInput")
    d_mi_w2 = nc.dram_tensor("mi_w2", [32, 2], f32, kind="ExternalInput")
    d_mi_b2 = nc.dram_tensor("mi_b2", [2], f32, kind="ExternalInput")
    d_dc_w1 = nc.dram_tensor("dc_w1", [H + 1, H], f32, kind="ExternalInput")
    d_dc_b1 = nc.dram_tensor("dc_b1", [H], f32, kind="ExternalInput")
    d_dc_w2 = nc.dram_tensor("dc_w2", [H, H], f32, kind="ExternalInput")
    d_dc_b2 = nc.dram_tensor("dc_b2", [H], f32, kind="ExternalInput")
    d_wv = nc.dram_tensor("wv", [H, H], f32, kind="ExternalInput")
    d_bv = nc.dram_tensor("bv", [H], f32, kind="ExternalInput")
    d_wo = nc.dram_tensor("wo", [H, H], f32, kind="ExternalInput")
    d_bo = nc.dram_tensor("bo", [H], f32, kind="ExternalInput")

    d_ident = nc.dram_tensor("ident128", [128, 128], f32, kind="ExternalInput")
    d_dT = nc.dram_tensor("dT_row", [B_CORE], f32, kind="ExternalInput")
    d_out = nc.dram_tensor("out", [B_CORE, 2 * H], bf16, kind="ExternalOutput")
    d_warm = nc.dram_tensor("warm", [128, TILE_N], f32, kind="ExternalOutput")

    with tile.TileContext(nc) as tc, ExitStack() as ctx:
        singles = ctx.enter_context(tc.tile_pool(name="singles", bufs=1))
        inp = ctx.enter_context(tc.tile_pool(name="inp", bufs=12))
        ps_tr = ctx.enter_context(tc.tile_pool(name="ps_tr", bufs=2, space="PSUM"))
        ps_mm = ctx.enter_context(tc.tile_pool(name="ps_mm", bufs=6, space="PSUM"))
        finp = ctx.enter_context(tc.tile_pool(name="finp", bufs=8))
        fintp = ctx.enter_context(tc.tile_pool(name="fintp", bufs=4))
        g1p = ctx.enter_context(tc.tile_pool(name="g1p", bufs=4))
        stp = ctx.enter_context(tc.tile_pool(name="stp", bufs=4))
        compp = ctx.enter_context(tc.tile_pool(name="compp", bufs=4))
        outp = ctx.enter_context(tc.tile_pool(name="outp", bufs=2))
        tmpp = ctx.enter_context(tc.tile_pool(name="tmpp", bufs=6))
        prol_ctx = ExitStack()
        prolp = prol_ctx.enter_context(tc.tile_pool(name="prolp", bufs=1))

        dT_bf = singles.tile([1, B_CORE], bf16, tag="dT_bf")
        Dball = singles.tile([128, B_CORE], bf16, tag="Dball")

        # ---------------- warmup: flip HAM to 8/8 early ---------------------
        # zero-weight matmuls with no input dependency; filler jobs keep the
        # PE HAM activity window busy through the whole prologue
        wtile = singles.tile([128, TILE_N], bf16, tag="wtile")
        nc.vector.memset(wtile, 0.0)
        ps_w = ps_mm.tile([128, TILE_N], f32, tag="mm", name="ps_w")
        for i in range(20):
            nc.tensor.matmul(ps_w, wtile[:, 0:128], wtile, start=(i == 0),
                             stop=False)

        def wfill_job(last=False):
            def emit():
                nc.tensor.matmul(ps_w, wtile[:, 0:128], wtile, start=False,
                                 stop=last)
                nc.tensor.matmul(ps_w, wtile[:, 0:128], wtile, start=last,
                                 stop=last)
                if last:
                    warm_sb = tmpp.tile([128, TILE_N], f32, tag="warm_sb",
                                        name="warm_sb", bufs=1)
                    nc.vector.tensor_copy(warm_sb, ps_w)
                    nc.sync.dma_start(out=d_warm[:, :], in_=warm_sb)
            return emit

        # ---------------- sync (HWDGE) loads --------------------------------
        qual = singles.tile([128, RC_TOT, 11], f32, tag="qual")
        nc.sync.dma_start(out=qual, in_=d_qual.rearrange("(c p) f -> p c f", p=128))
        mrm = singles.tile([128, RC_TOT], f32, tag="mrm")
        nc.sync.dma_start(out=mrm, in_=d_miss.rearrange("(c p) -> p c", p=128))

        dcb1 = singles.tile([128, PC], f32, tag="dcb1")
        nc.sync.dma_start(out=dcb1, in_=d_dc_b1.rearrange("(m p) -> p m", p=128))
        dcb2 = singles.tile([128, PC], f32, tag="dcb2")
        nc.sync.dma_start(out=dcb2, in_=d_dc_b2.rearrange("(m p) -> p m", p=128))

        # tiny MLP weights: f32 sync loads + on-chip bf16 casts
        tinyf = {}
        for nm, dten in [("qa_w1", d_qa_w1), ("qa_w2", d_qa_w2), ("qa_w3", d_qa_w3),
                         ("mi_w1", d_mi_w1), ("mi_w2", d_mi_w2)]:
            tf = prolp.tile(list(dten.shape), f32, tag=f"tf_{nm}", name=f"{nm}_f")
            nc.sync.dma_start(out=tf, in_=dten[:, :])
            tinyf[nm] = tf
        qaw1 = singles.tile([11, 64], bf16, tag="qaw1")
        nc.vector.tensor_copy(qaw1, tinyf["qa_w1"])
        qaw2 = singles.tile([64, 32], bf16, tag="qaw2")
        nc.vector.tensor_copy(qaw2, tinyf["qa_w2"])
        qaw3 = singles.tile([32, 1], bf16, tag="qaw3")
        nc.vector.tensor_copy(qaw3, tinyf["qa_w3"])
        miw1 = singles.tile([4, 32], bf16, tag="miw1")
        nc.vector.tensor_copy(miw1, tinyf["mi_w1"])
        wdiff = singles.tile([32, 1], bf16, tag="wdiff")
        nc.vector.tensor_sub(wdiff, tinyf["mi_w2"][:, 0:1], tinyf["mi_w2"][:, 1:2])

        qab1 = singles.tile([64, 1], f32, tag="qab1")
        nc.sync.dma_start(out=qab1, in_=d_qa_b1[:].unsqueeze(1))
        qab2 = singles.tile([32, 1], f32, tag="qab2")
        nc.sync.dma_start(out=qab2, in_=d_qa_b2[:].unsqueeze(1))
        qab3 = singles.tile([1, 1], f32, tag="qab3")
        nc.sync.dma_start(out=qab3, in_=d_qa_b3[:].unsqueeze(1))
        qab3h = singles.tile([1, 1], f32, tag="qab3h")
        nc.vector.tensor_scalar(qab3h, qab3, 0.5, None, OP.mult)
        mib1 = singles.tile([32, 1], f32, tag="mib1")
        nc.sync.dma_start(out=mib1, in_=d_mi_b1[:].unsqueeze(1))
        mib2f = singles.tile([1, 2], f32, tag="mib2f")
        nc.sync.dma_start(out=mib2f, in_=d_mi_b2[:].unsqueeze(0))
        db = singles.tile([1, 1], f32, tag="db")
        nc.vector.tensor_sub(db, mib2f[:, 0:1], mib2f[:, 1:2])
        nc.vector.tensor_scalar(db, db, 0.5, None, OP.mult)

        bo_sb = singles.tile([1, H], f32, tag="bo_sb")
        nc.sync.dma_start(out=bo_sb, in_=d_bo[:].unsqueeze(0))
        bvcol = singles.tile([128, PC], f32, tag="bvcol")
        nc.sync.dma_start(out=bvcol, in_=d_bv.rearrange("(k p) -> p k", p=128))
        ones_r = singles.tile([1, 128], bf16, tag="ones_r")
        nc.vector.memset(ones_r, 1.0)

        dcw1_lastf = prolp.tile([1, H], f32, tag="dcw1_lastf", name="dcw1_lastf")
        nc.sync.dma_start(out=dcw1_lastf, in_=d_dc_w1[H:H + 1, :])

        # ---------------- SWDGE cast loads (tile-0 features first) ----------
        feats = [d_img, d_eimg, d_txt, d_etxt]

        def emit_loads(t):
            tiles = []
            for dten in feats:
                it = inp.tile([128, PC, H], bf16, tag="in", name="it")
                nc.gpsimd.dma_start(
                    out=it,
                    in_=dten[t * TILE_N:(t + 1) * TILE_N, :].rearrange(
                        "(c p) f -> p c f", p=128))
                tiles.append(it)
            return tiles

        ident = singles.tile([128, 128], bf16, tag="ident")
        nc.gpsimd.dma_start(out=ident, in_=d_ident[:, :])
        nc.gpsimd.dma_start(out=dT_bf, in_=d_dT[:].unsqueeze(0))
        in_all = {0: emit_loads(0)}
        qual_bf = prolp.tile([128, RC_TOT, 11], bf16, tag="qual_bf", name="qual_bf")
        nc.gpsimd.dma_start(out=qual_bf, in_=d_qual.rearrange("(c p) f -> p c f", p=128))
        wv_sb = prolp.tile([128, PC, H], bf16, tag="wv_sb", name="wv_sb")
        nc.gpsimd.dma_start(out=wv_sb, in_=d_wv.rearrange("(c p) f -> p c f", p=128))
        wo_sb = prolp.tile([128, PC, H], bf16, tag="wo_sb", name="wo_sb")
        nc.gpsimd.dma_start(out=wo_sb, in_=d_wo.rearrange("(c p) f -> p c f", p=128))
        in_all[1] = emit_loads(1)

        # dc weights: chunked f32 sync loads -> fp8 x64 casts on ACT
        dcw1_8 = singles.tile([128, PC, H], fp8, tag="dcw1_8")
        dcw2_8 = singles.tile([128, PC, H], fp8, tag="dcw2_8")
        for dst8, dten in ((dcw1_8, d_dc_w1), (dcw2_8, d_dc_w2)):
            for k in range(PC):
                stg = prolp.tile([128, H], f32, tag="dcwst", name="stg", bufs=3)
                nc.sync.dma_start(out=stg, in_=dten[k * 128:(k + 1) * 128, :])
                nc.scalar.activation(dst8[:, k, :], stg, AF.Copy, scale=64.0)
        w1l64 = singles.tile([1, H], bf16, tag="w1l64")
        nc.vector.tensor_scalar(w1l64, dcw1_lastf, 64.0, None, OP.mult)

        # ---------------- exact fp32 per-row coefficient math ---------------
        def sc(tag):
            return singles.tile([128, RC_TOT], f32, tag=tag, name=tag)

        img_imp = qual[:, :, 6:7].rearrange("p c 1 -> p c")
        text_imp = qual[:, :, 7:8].rearrange("p c 1 -> p c")
        img_auth = qual[:, :, 8:9].rearrange("p c 1 -> p c")
        text_auth = qual[:, :, 9:10].rearrange("p c 1 -> p c")

        e0 = sc("e0"); e1 = sc("e1"); e2 = sc("e2")
        nc.vector.tensor_scalar(e0, mrm, 0.5, None, OP.is_lt)
        nc.vector.tensor_scalar(e1, mrm, 1.0, None, OP.is_equal)
        nc.vector.tensor_scalar(e2, mrm, 1.5, None, OP.is_gt)

        den = sc("den"); ratio = sc("ratio")
        nc.vector.scalar_tensor_tensor(den, img_imp, 1e-8, text_imp, OP.add, OP.add)
        nc.vector.reciprocal(den, den)
        nc.vector.tensor_mul(ratio, img_imp, den)
        ghi = sc("ghi"); glo = sc("glo"); si0 = sc("si0"); st0 = sc("st0")
        nc.vector.tensor_scalar(ghi, ratio, 0.6, None, OP.is_gt)
        nc.vector.tensor_scalar(glo, ratio, 0.4, None, OP.is_lt)
        nc.vector.tensor_sub(si0, ghi, glo)
        nc.vector.tensor_scalar(si0, si0, 0.1, 1.0, OP.mult, OP.add)
        nc.vector.tensor_scalar(st0, si0, -1.0, 2.0, OP.mult, OP.add)

        coef = singles.tile([128, RC_TOT, 6], f32, tag="coef")  # A_i B_i A_t B_t w_i w_t
        A_i = coef[:, :, 0:1].rearrange("p c 1 -> p c")
        B_i = coef[:, :, 1:2].rearrange("p c 1 -> p c")
        A_t = coef[:, :, 2:3].rearrange("p c 1 -> p c")
        B_t = coef[:, :, 3:4].rearrange("p c 1 -> p c")
        w_i = coef[:, :, 4:5].rearrange("p c 1 -> p c")
        w_t = coef[:, :, 5:6].rearrange("p c 1 -> p c")

        t_a = sc("t_a"); t_b = sc("t_b")
        nc.vector.scalar_tensor_tensor(t_a, img_auth, 0.3, e2, OP.mult, OP.mult)
        nc.vector.tensor_mul(t_b, si0, e0)
        nc.vector.tensor_add(t_a, t_a, t_b)
        nc.vector.tensor_add(A_i, t_a, e1)
        nc.vector.tensor_scalar(t_a, img_auth, -1.0, 1.0, OP.mult, OP.add)
        nc.vector.tensor_mul(t_a, t_a, img_imp)
        nc.vector.tensor_mul(B_i, t_a, e2)
        nc.vector.scalar_tensor_tensor(t_a, text_auth, 0.3, e1, OP.mult, OP.mult)
        nc.vector.tensor_mul(t_b, st0, e0)
        nc.vector.tensor_add(t_a, t_a, t_b)
        nc.vector.tensor_add(A_t, t_a, e2)
        nc.vector.tensor_scalar(t_a, text_auth, -1.0, 1.0, OP.mult, OP.add)
        nc.vector.tensor_mul(t_a, t_a, text_imp)
        nc.vector.tensor_mul(B_t, t_a, e1)

        # ---------------- persistent small tiles ---------------------------
        qualT = prolp.tile([11, B_CORE], bf16, tag="qualT", name="qualT")
        iaT = prolp.tile([4, B_CORE], bf16, tag="iaT", name="iaT")
        q_attT = prolp.tile([1, B_CORE], bf16, tag="q_attT", name="q_attT")
        img_wT = prolp.tile([1, B_CORE], bf16, tag="img_wT", name="img_wT")
        mlprm = singles.tile([128, RC_TOT, 2], f32, tag="mlprm")
        wc = singles.tile([128, PC, H], bf16, tag="wc")
        wvT = prolp.tile([128, PC, H], bf16, tag="wvT", name="wvT")
        if use_bvo:
            bvo = singles.tile([1, H], bf16, tag="bvo")

        # ---------------- prologue job lists --------------------------------
        def qual_tr_job(c):
            def emit():
                cs = slice(c * 128, (c + 1) * 128)
                pst = ps_tr.tile([128, 4, 128], bf16, tag="tr", name="pst")
                nc.tensor.transpose(pst[0:11, 0, :], qual_bf[:, c, :], ident)
                nc.tensor.transpose(pst[0:4, 1, :], qual_bf[:, c, 6:10], ident)
                nc.vector.tensor_copy(qualT[:, cs], pst[0:11, 0, :])
                nc.vector.tensor_copy(iaT[:, cs], pst[0:4, 1, :])
            return emit

        def bcast_job():
            def emit():
                nc.gpsimd.partition_broadcast(Dball, dT_bf)
            return emit

        def wvT_job(r):
            def emit():
                pst = ps_tr.tile([128, 4, 128], bf16, tag="tr", name="pst")
                for c4 in range(4):
                    nc.tensor.transpose(pst[:, c4, :],
                                        wv_sb[:, r, c4 * 128:(c4 + 1) * 128], ident)
                for c4 in range(4):
                    dst = wvT[:, c4, r * 128:(r + 1) * 128]
                    if c4 % 2 == 0:
                        nc.vector.tensor_copy(dst, pst[:, c4, :])
                    else:
                        nc.scalar.activation(dst, pst[:, c4, :], AF.Copy)
            return emit

        def wc_job(m):
            def emit():
                psw = ps_mm.tile([128, H], f32, tag="mm", name="psw")
                for k in range(PC):
                    nc.tensor.matmul(psw, wvT[:, k, m * 128:(m + 1) * 128],
                                     wo_sb[:, k, :], start=(k == 0),
                                     stop=(k == PC - 1))
                if m % 2 == 0:
                    nc.vector.tensor_copy(wc[:, m, :], psw)
                else:
                    nc.scalar.activation(wc[:, m, :], psw, AF.Copy)
            return emit

        def bvo_job():
            def emit():
                bvcol_b = tmpp.tile([128, PC], bf16, tag="bvcol_b", name="bvcol_b")
                nc.vector.tensor_copy(bvcol_b, bvcol)
                psb = ps_mm.tile([1, H], f32, tag="mm", name="psb")
                for k in range(PC):
                    nc.tensor.matmul(psb, bvcol_b[:, k:k + 1], wo_sb[:, k, :],
                                     start=(k == 0), stop=(k == PC - 1))
                nc.vector.tensor_add(bvo, psb, bo_sb)
            return emit

        def tiny_job(n):
            def emit():
                sl = slice(n * TILE_N, (n + 1) * TILE_N)
                ps1 = ps_mm.tile([64, TILE_N], f32, tag="mm", name="ps1")
                nc.tensor.matmul(ps1, qaw1, qualT[:, sl], start=True, stop=True)
                g1 = prolp.tile([64, TILE_N], bf16, tag="qg1", name="g1", bufs=2)
                nc.scalar.activation(g1, ps1, AF.Gelu, bias=qab1)
                ps2 = ps_mm.tile([32, TILE_N], f32, tag="mm", name="ps2")
                nc.tensor.matmul(ps2, qaw2, g1, start=True, stop=True)
                g2 = prolp.tile([32, TILE_N], bf16, tag="qg2", name="g2", bufs=2)
                nc.scalar.activation(g2, ps2, AF.Gelu, bias=qab2)
                ps3 = ps_mm.tile([1, TILE_N], f32, tag="mm", name="ps3")
                nc.tensor.matmul(ps3, qaw3, g2, start=True, stop=True)
                nc.scalar.activation(q_attT[:, sl], ps3, AF.Tanh, bias=qab3h,
                                     scale=0.5)
                psm1 = ps_mm.tile([32, TILE_N], f32, tag="mm", name="psm1")
                nc.tensor.matmul(psm1, miw1, iaT[:, sl], start=True, stop=True)
                mg = prolp.tile([32, TILE_N], bf16, tag="mg", name="mg", bufs=2)
                nc.scalar.activation(mg, psm1, AF.Gelu, bias=mib1)
                psm2 = ps_mm.tile([1, TILE_N], f32, tag="mm", name="psm2")
                nc.tensor.matmul(psm2, wdiff, mg, start=True, stop=True)
                nc.scalar.activation(img_wT[:, sl], psm2, AF.Tanh, bias=db,
                                     scale=0.5)
            return emit

        def gate_tr_job(c0):
            def emit():
                pst = ps_tr.tile([128, 4, 128], bf16, tag="tr", name="pst")
                for i, c in enumerate((c0, c0 + 1)):
                    cs = slice(c * 128, (c + 1) * 128)
                    nc.tensor.transpose(pst[:, 2 * i, 0:1], q_attT[:, cs],
                                        ident[0:1, 0:1])
                    nc.tensor.transpose(pst[:, 2 * i + 1, 0:1], img_wT[:, cs],
                                        ident[0:1, 0:1])
                for i, c in enumerate((c0, c0 + 1)):
                    nc.vector.tensor_copy(
                        mlprm[:, c, :],
                        pst[:, 2 * i:2 * i + 2, 0:1].rearrange("p a 1 -> p a"))
            return emit

        def gate_math_job():
            def emit():
                hq = mlprm[:, :, 0:1].rearrange("p c 1 -> p c")
                hw = mlprm[:, :, 1:2].rearrange("p c 1 -> p c")
                nc.vector.tensor_scalar(t_b, hw, 1.0, None, OP.add)
                nc.vector.scalar_tensor_tensor(w_i, hq, 1.0, t_b, OP.add, OP.mult)
                nc.vector.tensor_scalar(w_i, w_i, 0.25, None, OP.mult)
                nc.vector.tensor_scalar(t_b, hq, 0.5, 0.5, OP.mult, OP.add)
                nc.vector.tensor_sub(w_t, t_b, w_i)
            return emit

        # ---------------- per-tile phase jobs -------------------------------
        fin_specs = [(0, 1, A_i, B_i), (2, 3, A_t, B_t)]
        state = {}

        def get_state(t):
            if t not in state:
                state[t] = {
                    "fin": {},
                    "finT": {pi: fintp.tile([128, PC, TILE_N], bf16, tag="finT",
                                            name="fb") for pi in range(2)},
                    "finT8": {pi: fintp.tile([128, PC, TILE_N], fp8, tag="finT8",
                                             name="f8") for pi in range(2)},
                    "g1T": {pi: g1p.tile([128, PC, TILE_N], fp8, tag="g1",
                                         name="gt") for pi in range(2)},
                    "compT": {pi: compp.tile([128, PC, TILE_N], bf16, tag="comp",
                                             name="ct") for pi in range(2)},
                }
            return state[t]

        def load_job(t):
            def emit():
                in_all[t] = emit_loads(t)
            return emit

        def combine_job(t, pi, c):
            def emit():
                st_ = get_state(t)
                bfi, efi, Ac, Bc = fin_specs[pi]
                g = t * PC + c
                in_sb = in_all[t]
                tmp = tmpp.tile([128, H], bf16, tag="ctmp", name="tmp")
                nc.vector.tensor_scalar(tmp, in_sb[efi][:, c, :],
                                        Bc[:, g:g + 1], None, OP.mult)
                ft = finp.tile([128, H], bf16, tag="fin", name="ft")
                nc.vector.scalar_tensor_tensor(ft, in_sb[bfi][:, c, :],
                                               Ac[:, g:g + 1], tmp,
                                               OP.mult, OP.add)
                st_["fin"][(pi, c)] = ft
            return emit

        def transpose_job(t, pi, fc):
            def emit():
                st_ = get_state(t)
                pst = ps_tr.tile([128, 4, 128], bf16, tag="tr", name="pst")
                for c in range(PC):
                    nc.tensor.transpose(
                        pst[:, c, :],
                        st_["fin"][(pi, c)][:, fc * 128:(fc + 1) * 128], ident)
                nc.vector.tensor_copy(st_["finT"][pi][:, fc, :], pst)
                nc.scalar.activation(st_["finT8"][pi][:, fc, :], pst, AF.Copy)
            return emit

        def z1_job(t, m):
            def emit():
                st_ = get_state(t)
                tsl = slice(t * TILE_N, (t + 1) * TILE_N)
                ms = slice(m * 128, (m + 1) * 128)
                zps = {pi: ps_mm.tile([128, TILE_N], f32, tag="mm", name="z1")
                       for pi in range(2)}
                for j in range(2):
                    for pi in range(2):
                        nc.tensor.matmul(zps[pi], dcw1_8[:, 2 * j:2 * j + 2, ms],
                                         st_["finT8"][pi][:, 2 * j:2 * j + 2, :],
                                         start=(j == 0), stop=False,
                                         perf_mode=DR)
                for pi in range(2):
                    nc.tensor.matmul(zps[pi], w1l64[:, ms], dT_bf[:, tsl],
                                     start=False, stop=True)
                for pi in range(2):
                    nc.scalar.activation(st_["g1T"][pi][:, m, :], zps[pi], AF.Gelu,
                                         bias=dcb1[:, m:m + 1], scale=1.0 / 64.0)
            return emit

        def z2_job(t, m):
            def emit():
                st_ = get_state(t)
                tsl = slice(t * TILE_N, (t + 1) * TILE_N)
                Db = Dball[:, tsl]
                zps = {pi: ps_mm.tile([128, TILE_N], f32, tag="mm", name="z2")
                       for pi in range(2)}
                for j in range(2):
                    for pi in range(2):
                        nc.tensor.matmul(zps[pi], dcw2_8[:, 2 * j:2 * j + 2,
                                                         m * 128:(m + 1) * 128],
                                         st_["g1T"][pi][:, 2 * j:2 * j + 2, :],
                                         start=(j == 0), stop=(j == 1),
                                         perf_mode=DR)
                for pi in range(2):
                    st = stp.tile([128, TILE_N], bf16, tag="sT", name="st")
                    nc.scalar.activation(st, zps[pi], AF.Sigmoid,
                                         bias=dcb2[:, m:m + 1], scale=1.0 / 64.0)
                    t1 = tmpp.tile([128, TILE_N], bf16, tag="t1", name="t1")
                    nc.vector.tensor_mul(t1, st, Db)
                    nc.vector.scalar_tensor_tensor(st_["compT"][pi][:, m, :], t1,
                                                   1.0, st_["finT"][pi][:, m, :],
                                                   OP.add, OP.mult)
            return emit

        def att_jobs(t):
            stage = outp.tile([128, PC, 2 * H], bf16, tag="stage", name="stage")
            jobs = []

            def att_one(srcp, wcol, ocol, r):
                def emit():
                    st_ = get_state(t)
                    g = t * PC + r
                    att = ps_mm.tile([128, H], f32, tag="mm", name="att")
                    for k in range(PC):
                        nc.tensor.matmul(
                            att, st_["compT"][srcp][:, k, r * 128:(r + 1) * 128],
                            wc[:, k, :], start=(k == 0),
                            stop=(not use_bvo and k == PC - 1))
                    if use_bvo:
                        nc.tensor.matmul(att, ones_r, bvo, start=False, stop=True)
                    nc.scalar.activation(stage[:, r, ocol * H:(ocol + 1) * H], att,
                                         AF.Copy, scale=wcol[:, g:g + 1])
                return emit

            for srcp, wcol, ocol in [(1, w_i, 0), (0, w_t, 1)]:
                for r in range(PC):
                    jobs.append(att_one(srcp, wcol, ocol, r))

            def dma_out():
                tsl = slice(t * TILE_N, (t + 1) * TILE_N)
                nc.sync.dma_start(
                    out=d_out[tsl, :].rearrange("(c p) f -> p c f", p=128),
                    in_=stage)
                del state[t]
            jobs.append(dma_out)
            return jobs

        def jobs_A(t):
            jobs = []
            for pi in range(2):
                for c in range(PC):
                    jobs.append(combine_job(t, pi, c))
            for pi in range(2):
                for fc in range(PC):
                    jobs.append(transpose_job(t, pi, fc))
            return jobs

        def jobs_B(t):
            return [z1_job(t, m) for m in range(PC)] + \
                   [z2_job(t, m) for m in range(PC)]

        # ---------------- pipelined emission --------------------------------
        # step -1: A(0) + wc build + quality transposes
        prol2 = ([bcast_job()] +
                 [wvT_job(r) for r in range(PC)] +
                 [wc_job(m) for m in range(PC)] +
                 ([bvo_job()] if use_bvo else []))
        wfill1 = [wfill_job() for _ in range(10)]
        for job in _interleave(jobs_A(0), prol2, wfill1):
            job()

        # step 0: B(0) x A(1) + quality/tiny/gates + loads(2)
        prol3 = ([qual_tr_job(c) for c in range(RC_TOT)] +
                 [tiny_job(n) for n in range(N_TILES)] +
                 [gate_tr_job(c0) for c0 in range(0, RC_TOT, 2)] +
                 [gate_math_job()])
        wfill2 = [wfill_job() for _ in range(6)]
        for job in _interleave([load_job(2)] + jobs_B(0), jobs_A(1), prol3,
                               wfill2):
            job()

        prol_ctx.close()

        # steps 1..3: C(t-1) x B(t) x A(t+1)
        for t in range(1, N_TILES):
            ls = [att_jobs(t - 1), jobs_B(t)]
            if t + 1 < N_TILES:
                a = jobs_A(t + 1)
                if t + 2 < N_TILES:
                    a = [load_job(t + 2)] + a
                ls.append(a)
            if t < N_TILES - 1:
                ls.append([wfill_job() for _ in range(3)])
            else:
                ls.append([wfill_job() for _ in range(2)] +
                          [wfill_job(last=True)])
            for job in _interleave(*ls):
                job()

        # final: C(3)
        for job in att_jobs(N_TILES - 1):
            job()

    nc.compile()
    _dedupe_ldweights(nc, mybir)
    return nc


def _dedupe_ldweights(nc, mybir):
    """Drop InstLdweights that reload the exact weights already resident in
    the PE array (no intervening loads). Only sync-free LDWs are removed."""
    removed = 0
    for blk in nc.m.functions[0].blocks:
        insts = list(blk.instructions)
        keep = []
        cur = None
        for i in insts:
            if getattr(i, 'engine', None) != mybir.EngineType.PE:
                keep.append(i)
                continue
            t = type(i).__name__
            if t == 'InstLdweights':
                ap = i.ins[0]
                key = (str(ap.memref), ap.offset, str(ap.ap), str(ap.dtype),
                       bool(getattr(i, 'is_transpose', False)),
                       str(getattr(i, 'perf_mode', None)),
                       str(getattr(i, 'tile_position', None)))
                si = i.sync_info
                has_sync = bool(si and (si.on_wait or si.on_update))
                if key == cur and not has_sync:
                    removed += 1
                    continue
                cur = key
                keep.append(i)
            elif t == 'InstMatmult':
                keep.append(i)
            else:
                cur = None
                keep.append(i)
        if removed:
            blk.instructions = keep
    return removed


def _get_program(use_bvo=True):
    key = ("nc", use_bvo)
    if key not in _CACHE:
        _CACHE[key] = _build_program(use_bvo)
    return _CACHE[key]


def kernel(**inputs) -> np.ndarray:
    global last_exec_time_ns, last_trace_path, last_scope_times
    import os
    from concourse.bass_utils import run_bass_kernel_spmd

    # value-specialize: the v/o projection biases are zero in this problem's
    # setup_inputs, making the rank-1 bias accumulation a no-op
    use_bvo = bool(np.any(np.asarray(inputs["bv"])) or
                   np.any(np.asarray(inputs["bo"])))
    nc = _get_program(use_bvo)

    full = {k: np.ascontiguousarray(np.asarray(v, dtype=np.float32))
            for k, v in inputs.items() if k != "missing_type"}
    missing_f = np.ascontiguousarray(
        np.asarray(inputs["missing_type"]).astype(np.float32))

    per_core_keys = ["image_feat", "text_feat", "enhanced_image_feat",
                     "enhanced_text_feat", "quality"]
    weight_keys = ["qa_w1", "qa_b1", "qa_w2", "qa_b2", "qa_w3", "qa_b3",
                   "mi_w1", "mi_b1", "mi_w2", "mi_b2",
                   "dc_w1", "dc_b1", "dc_w2", "dc_b2",
                   "wv", "bv", "wo", "bo"]

    ident128 = np.eye(128, dtype=np.float32)
    dT_row = np.ascontiguousarray(full["quality"][:, 10])
    in_maps = []
    for c in range(N_CORES):
        sl = slice(c * B_CORE, (c + 1) * B_CORE)
        m = {k: full[k][sl] for k in per_core_keys}
        m["missing_f"] = missing_f[sl]
        m["ident128"] = ident128
        m["dT_row"] = dT_row[sl]
        for k in weight_keys:
            m[k] = full[k]
        in_maps.append(m)

    trace = os.environ.get("KERNEL_TRACE", "0") == "1"
    res = run_bass_kernel_spmd(nc, in_maps, core_ids=list(range(N_CORES)),
                               trace=trace)
    last_exec_time_ns = res.exec_time_ns
    last_scope_times = res.per_core_scope_times
    if res.instructions_and_trace is not None:
        last_trace_path = res.instructions_and_trace[1]

    out = np.empty((B_FULL, 2 * H), dtype=np.float32)
    for c in range(N_CORES):
        out[c * B_CORE:(c + 1) * B_CORE] = res.results[c]["out"].astype(np.float32)
    return out
